# revision 34
# baseline (speedup 1.0000x reference)
"""CIF (continuous integrate-and-fire) layer as a Trainium2 Bass/Tile kernel.

Strategy
--------
Data-parallel over batch: B=16 sequences, 8 NeuronCores, 2 sequences/core.

The CIF scatter is reformulated as a dense banded matmul: sequence step s
carries mass over the interval [csum[s-1], csum[s]] and output slot t
integrates mass over [t, t+1), so the weight of x[s] on out[t] is the
interval overlap

    W[s, t] = clip(csum[s] - t, 0, 1) - clip(csum[s-1] - t, 0, 1)

which matches the reference's floor/scatter/cumsum formulation exactly for
all t < T (the reference's dump slots t >= T are dropped by out[:, :T]).
Then out_b = W_b^T @ x_b is a tensor-engine matmul.

Per-core pipeline (per sequence):
  1. xT chunks (128c x 512s, bf16) as plain DMA loads from a host-
     transposed (C, S) copy of x, issued per (s-block, c-tile) so mm1
     starts as soon as the first s-block's tiles land
  2. hT = tanh(w1^T @ xT + b1)       [PE + ACT, bf16]
  3. logits row (1, 512) per s-block [PE]; sigmoid+b2 [ACT]
  4. columnize alpha into (128, 16) via K=1 outer-product matmuls,
     overlapped per s-block
  5. all alpha math in 128-lane column space: mask (pre-columnized on
     host), alpha_sum via ones-matmul partition broadcast, scale,
     per-tile prefix via free-dim scan, cumsum via triangular matmul
  6. W tiles (128s x 512t) on DVE (min/max/sub) + relu-cast bf16 on ACT
  7. out (t, c) psum = sum_s W^T x   [PE, tt-outer so copies overlap]
"""

import os
import sys

import numpy as np

for _p in ("/root/.axon_site", "/root/.axon_site/_ro/trn_rl_repo",
           "/root/.axon_site/_ro/pypackages", "/opt/trn_rl_repo"):
    if os.path.isdir(_p) and _p not in sys.path:
        sys.path.append(_p)

import ml_dtypes  # noqa: E402

# The agent image's antenv package lacks axon_hooks; run_bass_kernel_spmd
# imports it when BASS_TRACE is set. Register a working shim so tracing
# degrades gracefully (and works when the libaxon profile ABI is present).
try:
    import antenv.axon_hooks  # noqa: F401
except ImportError:
    try:
        import types

        import antenv
        from trn_agent_boot.trn_boot import _ntff_profile_via_ctypes

        _hooks = types.ModuleType("antenv.axon_hooks")
        _hook_holder = [None]
        try:
            _hook_holder[0] = _ntff_profile_via_ctypes(
                "/opt/axon/libaxon_pjrt.so")
        except Exception:
            pass
        _hooks.get_axon_ntff_profile_hook = lambda: _hook_holder[0]
        _hooks.set_axon_ntff_profile_hook = (
            lambda h: _hook_holder.__setitem__(0, h))
        sys.modules["antenv.axon_hooks"] = _hooks
        antenv.axon_hooks = _hooks
    except Exception:
        pass

import concourse.bass as bass  # noqa: E402
from concourse import bacc  # noqa: E402
import concourse.tile as tile  # noqa: E402
from concourse import tile_rust  # noqa: E402
from concourse import mybir  # noqa: E402
from concourse.bass_utils import run_bass_kernel_spmd  # noqa: E402

B, S, C, H, T = 16, 2048, 1024, 1024, 512
BETA = 1.0
NCORES = 8
BPC = B // NCORES            # sequences per core = 2
P = 128
ST = S // P                  # s-tiles per sequence = 16
SB = S // 512                # 512-wide s-blocks = 4
CT = C // P                  # c-tiles = 8
HT = H // P                  # h-tiles = 8
TT = T // P                  # t-tiles = 4

F32 = mybir.dt.float32
BF16 = mybir.dt.bfloat16
OP = mybir.AluOpType
ACTF = mybir.ActivationFunctionType

_PROGRAM = None
LAST_RESULTS = None


def _build_program():
    nc = bacc.Bacc()

    xbf = nc.declare_dram_parameter("xbf", [BPC, S, C], BF16, isOutput=False)
    xcs = nc.declare_dram_parameter("xcs", [BPC, C, S], BF16, isOutput=False)
    w1bf = nc.declare_dram_parameter("w1bf", [C, H], BF16, isOutput=False)
    b1d = nc.declare_dram_parameter("b1d", [H], F32, isOutput=False)
    w2bf = nc.declare_dram_parameter("w2bf", [H], BF16, isOutput=False)
    b2d = nc.declare_dram_parameter("b2d", [1, 1], F32, isOutput=False)
    maskc = nc.declare_dram_parameter("maskc", [BPC, P, ST], F32, isOutput=False)
    desired = nc.declare_dram_parameter("desired", [P, BPC], F32, isOutput=False)

    out_d = nc.declare_dram_parameter("out", [T, BPC, C], F32, isOutput=True)
    asum_d = nc.declare_dram_parameter("alpha_sum", [1, BPC], F32, isOutput=True)

    with tile.TileContext(nc) as tc:
        with (
            tc.tile_pool(name="consts", bufs=1) as consts,
            tc.tile_pool(name="xT", bufs=4 * CT) as p_xT,
            tc.tile_pool(name="xsc", bufs=2) as p_xsc,
            tc.tile_pool(name="hT", bufs=2) as p_hT,
            tc.tile_pool(name="wmat", bufs=BPC) as p_W,
            tc.tile_pool(name="rows", bufs=1) as p_rows,
            tc.tile_pool(name="cols", bufs=2) as p_cols,
            tc.tile_pool(name="wtmp", bufs=3) as p_wtmp,
            tc.tile_pool(name="ostage", bufs=3) as p_ost,
            tc.tile_pool(name="ps_mm1", bufs=2, space="PSUM") as ps_mm1,
            tc.tile_pool(name="ps_row", bufs=1, space="PSUM") as ps_row,
            tc.tile_pool(name="ps_small", bufs=2, space="PSUM") as ps_small,
            tc.tile_pool(name="ps_mm2", bufs=3, space="PSUM") as ps_mm2,
        ):
            # ---- w1 via the Scalar HWDGE queue; the Sync HWDGE queue
            # carries only the xT chunk loads (keeps mm1's feed clear) ---
            w1_sb = consts.tile([P, CT, H], BF16)          # [c_p, c_tile, h]
            w1_re = w1bf.rearrange("(ct p) h -> p ct h", p=P)
            for ct in range(CT):
                nc.scalar.dma_start(w1_sb[:, ct:ct + 1, :],
                                    w1_re[:, ct:ct + 1, :])

            # ---- small constants on the SWDGE queue (off the Sync FIFO) --
            b1_sb = consts.tile([P, HT], F32)              # [h_p, h_tile]
            nc.gpsimd.dma_start(b1_sb, b1d.rearrange("(ht p) -> p ht", p=P))
            w2_sb = consts.tile([P, HT], BF16)
            nc.gpsimd.dma_start(w2_sb, w2bf.rearrange("(ht p) -> p ht", p=P))
            b2_sb = consts.tile([1, 1], F32)
            nc.gpsimd.dma_start(b2_sb, b2d[:, :])
            des_sb = consts.tile([P, BPC], F32)
            nc.gpsimd.dma_start(des_sb, desired[:, :])
            mask_sb = consts.tile([P, BPC, ST], F32)
            nc.gpsimd.dma_start(mask_sb, maskc.rearrange("b p j -> p b j"))

            iota0 = consts.tile([P, 512], F32)             # row = 0..511
            nc.gpsimd.iota(iota0, pattern=[[1, 512]], base=0,
                           channel_multiplier=0,
                           allow_small_or_imprecise_dtypes=True)
            iota1 = consts.tile([P, 512], F32)             # row = 1..512
            nc.gpsimd.iota(iota1, pattern=[[1, 512]], base=1,
                           channel_multiplier=0,
                           allow_small_or_imprecise_dtypes=True)

            # LT[k, m] = 1.0 iff k <= m  (inclusive lower-tri, csum matmul)
            pidx = consts.tile([P, 1], F32)                # pidx[p] = p
            nc.gpsimd.iota(pidx, pattern=[[1, 1]], base=0,
                           channel_multiplier=1,
                           allow_small_or_imprecise_dtypes=True)
            lt_sb = consts.tile([P, P], F32)
            nc.vector.tensor_scalar(lt_sb, iota0[:, :P], pidx, None, OP.is_ge)
            ones128 = consts.tile([P, P], F32)
            nc.vector.memset(ones128, 1.0)
            one_11 = consts.tile([1, 1], F32)
            nc.vector.memset(one_11, 1.0)
            zeros_c = consts.tile([P, ST], F32)
            nc.vector.memset(zeros_c, 0.0)

            csum_cols = []
            prev_cols = []
            last_xt_inst = None

            # ================= phase 1: alpha / csum per sequence ========
            for b in range(BPC):
                # xT chunks (c_p=128, 512s): plain loads from the host-
                # transposed (C, S) copy of x -- s-block major for dep grain
                xTs = {}
                for sb in range(SB):
                    for ct in range(CT):
                        t_ = p_xT.tile([P, 512], BF16, tag="xT",
                                       name=f"xT_{b}_{sb}_{ct}")
                        last_xt_inst = nc.sync.dma_start(
                            t_, xcs[b, ct * P:(ct + 1) * P,
                                    sb * 512:(sb + 1) * 512])
                        xTs[(sb, ct)] = t_

                alpha_row = p_rows.tile([1, S], F32, tag=f"alpha_row{b}")
                aps = ps_small.tile([P, ST], F32, tag="smallps", name=f"aps{b}")

                for sb in range(SB):
                    hT_sb = p_hT.tile([P, HT, 512], BF16, tag="hT")
                    for ht in range(HT):
                        ps = ps_mm1.tile([P, 512], F32, tag="mm1")
                        for ct in range(CT):
                            nc.tensor.matmul(
                                ps,
                                w1_sb[:, ct, ht * P:(ht + 1) * P],
                                xTs[(sb, ct)][:, :],
                                start=(ct == 0), stop=(ct == CT - 1),
                            )
                        nc.scalar.activation(hT_sb[:, ht, :], ps, ACTF.Tanh,
                                             bias=b1_sb[:, ht:ht + 1])
                    lps = ps_row.tile([1, 512], F32, tag="logit")
                    for ht in range(HT):
                        nc.tensor.matmul(
                            lps, w2_sb[:, ht:ht + 1], hT_sb[:, ht, :],
                            start=(ht == 0), stop=(ht == HT - 1),
                        )
                    ssl = bass.ts(sb, 512)
                    nc.scalar.activation(alpha_row[:, ssl], lps, ACTF.Sigmoid,
                                         bias=b2_sb[0:1, 0:1])
                    # columnize this s-block's 4 s-tiles (K=1 outer products)
                    for j in range(4 * sb, 4 * sb + 4):
                        nc.tensor.matmul(aps[:, j:j + 1],
                                         alpha_row[0:1, j * P:(j + 1) * P],
                                         one_11, start=True, stop=True)

                # ---- column space: mask, sum, scale, cumsum ----
                araw = p_cols.tile([P, ST], F32, tag="araw")
                nc.vector.tensor_mul(araw, aps, mask_sb[:, b, :])
                apart = p_cols.tile([P, 1], F32, tag="apart")
                nc.vector.tensor_reduce(apart, araw,
                                        axis=mybir.AxisListType.X, op=OP.add)
                sps = ps_small.tile([P, 1], F32, tag="smallps", name=f"sps{b}")
                nc.tensor.matmul(sps, ones128, apart, start=True, stop=True)
                asum = p_cols.tile([P, 1], F32, tag="asum")
                nc.vector.tensor_copy(asum, sps)
                nc.gpsimd.dma_start(asum_d[0:1, b:b + 1], asum[0:1, 0:1])
                rec = p_cols.tile([P, 1], F32, tag="rec")
                nc.vector.reciprocal(rec, asum)
                scl = p_cols.tile([P, 1], F32, tag="scl")
                nc.vector.tensor_mul(scl, rec, des_sb[:, b:b + 1])
                a_scl = p_cols.tile([P, ST], F32, tag="a_scl")
                nc.vector.tensor_scalar(a_scl, araw, scl, None, OP.mult)

                # inter-tile exclusive prefix (per partition, along tiles)
                bincl = p_cols.tile([P, ST], F32, tag="bincl")
                nc.vector.tensor_tensor_scan(bincl, a_scl, zeros_c, 0.0,
                                             OP.add, OP.add)
                bexcl = p_cols.tile([P, ST], F32, tag="bexcl")
                nc.vector.tensor_sub(bexcl, bincl, a_scl)

                # csum[p,j] = sum_{k<=p} a[k,j] + sum_k bexcl[k,j]
                cps = ps_small.tile([P, ST], F32, tag="smallps", name=f"cps{b}")
                nc.tensor.matmul(cps, lt_sb, a_scl, start=True, stop=False)
                nc.tensor.matmul(cps, ones128, bexcl, start=False, stop=True)
                csum_c = p_cols.tile([P, ST], F32, tag="csum_c")
                nc.vector.tensor_copy(csum_c, cps)
                prev_c = p_cols.tile([P, ST], F32, tag="prev_c")
                nc.vector.tensor_sub(prev_c, csum_c, a_scl)
                csum_cols.append(csum_c)
                prev_cols.append(prev_c)

            # ================= phase 2: W build + output matmul ==========
            for b in range(BPC):
                w_t = p_W.tile([P, ST, T], BF16, tag="W")
                for j in range(ST):
                    m2 = p_wtmp.tile([P, T], F32, tag="m2")
                    nc.vector.tensor_scalar(m2, iota0[:, :T],
                                            prev_cols[b][:, j:j + 1], None,
                                            OP.max)
                    wd = p_wtmp.tile([P, T], F32, tag="wd")
                    nc.vector.scalar_tensor_tensor(
                        wd, iota1[:, :T], csum_cols[b][:, j:j + 1], m2,
                        OP.min, OP.subtract)
                    nc.scalar.activation(w_t[:, j, :], wd, ACTF.Relu)

                for ch in range(2):
                    csl = bass.ts(ch, 512)
                    x_sc = p_xsc.tile([P, ST, 512], BF16, tag="x_sc")
                    xsc_inst = nc.gpsimd.dma_start(
                        x_sc,
                        xbf[b, :, csl].rearrange("(j p) c -> p j c", p=P))
                    if last_xt_inst is not None:
                        # keep x_sc traffic off the DMA queues until the
                        # mm1-feeding xT loads have been issued
                        tile_rust.add_dep_helper(
                            xsc_inst.ins, last_xt_inst.ins,
                            sync=True,
                            reason="delay x_sc until xT loads queued")
                    for tt in range(TT):
                        pso = ps_mm2.tile([P, 512], F32, tag="mm2",
                                          name=f"mm2_{b}_{ch}_{tt}")
                        for j in range(ST):
                            nc.tensor.matmul(
                                pso,
                                w_t[:, j, tt * P:(tt + 1) * P],
                                x_sc[:, j, :],
                                start=(j == 0), stop=(j == ST - 1),
                            )
                        ost = p_ost.tile([P, 512], F32, tag="ost")
                        nc.vector.tensor_copy(ost, pso)
                        nc.gpsimd.dma_start(
                            out_d[tt * P:(tt + 1) * P, b, csl], ost)

    nc.finalize()
    return nc


def _get_program():
    global _PROGRAM
    if _PROGRAM is None:
        _PROGRAM = _build_program()
    return _PROGRAM


def kernel(x, encoder_padding_mask, target_lengths, w1, b1, v2, g2, b2):
    global LAST_RESULTS
    bf = ml_dtypes.bfloat16

    x = np.asarray(x)
    mask = np.asarray(encoder_padding_mask)
    tl = np.asarray(target_lengths)
    w1 = np.asarray(w1, dtype=np.float32)
    b1 = np.asarray(b1, dtype=np.float32)
    v2 = np.asarray(v2, dtype=np.float32)
    g2 = np.asarray(g2, dtype=np.float32)
    b2 = np.asarray(b2, dtype=np.float32)

    # host-side marshaling: device layouts / dtypes
    xb = np.ascontiguousarray(x.transpose(1, 0, 2)).astype(bf)     # (B,S,C)
    xcs = np.ascontiguousarray(xb.transpose(0, 2, 1))              # (B,C,S)
    w1bf = w1.astype(bf)
    w2 = (np.float32(g2.reshape(())) * v2 /
          np.float32(np.linalg.norm(v2))).astype(np.float32)
    w2bf = w2.astype(bf)
    b2a = np.asarray(b2, np.float32).reshape(1, 1)
    keepf = (~mask.astype(bool)).astype(np.float32)                # (B,S)
    # pre-columnized keep-mask: maskc[b, p, j] = keepf[b, j*128 + p]
    maskc = np.ascontiguousarray(
        keepf.reshape(B, ST, P).transpose(0, 2, 1))                # (B,P,ST)
    des = (np.float32(BETA) * tl.astype(np.float32) +
           np.float32(1e-4)).astype(np.float32)                    # (B,)

    nc = _get_program()
    in_maps = []
    for i in range(NCORES):
        sl = slice(i * BPC, (i + 1) * BPC)
        in_maps.append(dict(
            xbf=np.ascontiguousarray(xb[sl]),
            xcs=np.ascontiguousarray(xcs[sl]),
            w1bf=w1bf,
            b1d=b1,
            w2bf=w2bf,
            b2d=b2a,
            maskc=np.ascontiguousarray(maskc[sl]),
            desired=np.ascontiguousarray(
                np.broadcast_to(des[sl], (P, BPC))),
        ))

    res = run_bass_kernel_spmd(nc, in_maps, list(range(NCORES)))
    LAST_RESULTS = res

    out = np.concatenate([res.results[i]["out"] for i in range(NCORES)],
                         axis=1)                                   # (T,B,C)
    alpha_sum = np.concatenate(
        [res.results[i]["alpha_sum"].reshape(BPC) for i in range(NCORES)])
    feat_lengths = tl
    return out, feat_lengths, alpha_sum.astype(np.float32)


# revision 35
# speedup vs baseline: 1.2341x; 1.2341x over previous
"""CIF (continuous integrate-and-fire) layer as a Trainium2 Bass/Tile kernel.

Strategy
--------
Data-parallel over batch: B=16 sequences, 8 NeuronCores, 2 sequences/core.

The CIF scatter is reformulated as a dense banded matmul: sequence step s
carries mass over the interval [csum[s-1], csum[s]] and output slot t
integrates mass over [t, t+1), so the weight of x[s] on out[t] is the
interval overlap

    W[s, t] = clip(csum[s] - t, 0, 1) - clip(csum[s-1] - t, 0, 1)

which matches the reference's floor/scatter/cumsum formulation exactly for
all t < T (the reference's dump slots t >= T are dropped by out[:, :T]).
Then out_b = W_b^T @ x_b is a tensor-engine matmul.

Per-core pipeline (per sequence):
  1. xT chunks (128c x 512s, bf16) as plain DMA loads from a host-
     transposed (C, S) copy of x, issued per (s-block, c-tile) so mm1
     starts as soon as the first s-block's tiles land
  2. hT = tanh(w1^T @ xT + b1)       [PE + ACT, bf16]
  3. logits row (1, 512) per s-block [PE]; sigmoid+b2 [ACT]
  4. columnize alpha into (128, 16) via K=1 outer-product matmuls,
     overlapped per s-block
  5. all alpha math in 128-lane column space: mask (pre-columnized on
     host), alpha_sum via ones-matmul partition broadcast, scale,
     per-tile prefix via free-dim scan, cumsum via triangular matmul
  6. W tiles (128s x 512t) on DVE (min/max/sub) + relu-cast bf16 on ACT
  7. out (t, c) psum = sum_s W^T x   [PE, tt-outer so copies overlap]
"""

import os
import sys

import numpy as np

for _p in ("/root/.axon_site", "/root/.axon_site/_ro/trn_rl_repo",
           "/root/.axon_site/_ro/pypackages", "/opt/trn_rl_repo"):
    if os.path.isdir(_p) and _p not in sys.path:
        sys.path.append(_p)

import ml_dtypes  # noqa: E402

# The agent image's antenv package lacks axon_hooks; run_bass_kernel_spmd
# imports it when BASS_TRACE is set. Register a working shim so tracing
# degrades gracefully (and works when the libaxon profile ABI is present).
try:
    import antenv.axon_hooks  # noqa: F401
except ImportError:
    try:
        import types

        import antenv
        from trn_agent_boot.trn_boot import _ntff_profile_via_ctypes

        _hooks = types.ModuleType("antenv.axon_hooks")
        _hook_holder = [None]
        try:
            _hook_holder[0] = _ntff_profile_via_ctypes(
                "/opt/axon/libaxon_pjrt.so")
        except Exception:
            pass
        _hooks.get_axon_ntff_profile_hook = lambda: _hook_holder[0]
        _hooks.set_axon_ntff_profile_hook = (
            lambda h: _hook_holder.__setitem__(0, h))
        sys.modules["antenv.axon_hooks"] = _hooks
        antenv.axon_hooks = _hooks
    except Exception:
        pass

import concourse.bass as bass  # noqa: E402
from concourse import bacc  # noqa: E402
import concourse.tile as tile  # noqa: E402
from concourse import tile_rust  # noqa: E402
from concourse import mybir  # noqa: E402
from concourse.bass_utils import run_bass_kernel_spmd  # noqa: E402

B, S, C, H, T = 16, 2048, 1024, 1024, 512
BETA = 1.0
NCORES = 8
BPC = B // NCORES            # sequences per core = 2
P = 128
ST = S // P                  # s-tiles per sequence = 16
SB = S // 512                # 512-wide s-blocks = 4
CT = C // P                  # c-tiles = 8
HT = H // P                  # h-tiles = 8
TT = T // P                  # t-tiles = 4

F32 = mybir.dt.float32
BF16 = mybir.dt.bfloat16
OP = mybir.AluOpType
ACTF = mybir.ActivationFunctionType

_PROGRAM = None
LAST_RESULTS = None


def _build_program():
    nc = bacc.Bacc()

    xbf = nc.declare_dram_parameter("xbf", [BPC, S, C], BF16, isOutput=False)
    xcs = nc.declare_dram_parameter("xcs", [BPC, C, S], BF16, isOutput=False)
    w1bf = nc.declare_dram_parameter("w1bf", [C, H], BF16, isOutput=False)
    b1d = nc.declare_dram_parameter("b1d", [P, H], F32, isOutput=False)
    w2bf = nc.declare_dram_parameter("w2bf", [P, H], BF16, isOutput=False)
    b2d = nc.declare_dram_parameter("b2d", [P, 1], F32, isOutput=False)
    maskc = nc.declare_dram_parameter("maskc", [BPC, P, ST], F32, isOutput=False)
    desired = nc.declare_dram_parameter("desired", [P, BPC], F32, isOutput=False)

    out_d = nc.declare_dram_parameter("out", [T, BPC, C], F32, isOutput=True)
    asum_d = nc.declare_dram_parameter("alpha_sum", [1, BPC], F32, isOutput=True)

    with tile.TileContext(nc) as tc:
        with (
            tc.tile_pool(name="consts", bufs=1) as consts,
            tc.tile_pool(name="xT", bufs=4 * CT) as p_xT,
            tc.tile_pool(name="xsc", bufs=2) as p_xsc,
            tc.tile_pool(name="hT", bufs=2) as p_hT,
            tc.tile_pool(name="wmat", bufs=BPC) as p_W,
            tc.tile_pool(name="cols", bufs=2) as p_cols,
            tc.tile_pool(name="wtmp", bufs=3) as p_wtmp,
            tc.tile_pool(name="hpre", bufs=3) as p_hpre,
            tc.tile_pool(name="scr", bufs=2) as p_scr,
            tc.tile_pool(name="ostage", bufs=3) as p_ost,
            tc.tile_pool(name="ps_mm1", bufs=3, space="PSUM") as ps_mm1,
            tc.tile_pool(name="ps_small", bufs=2, space="PSUM") as ps_small,
            tc.tile_pool(name="ps_mm2", bufs=3, space="PSUM") as ps_mm2,
        ):
            # ---- w1 via the Scalar HWDGE queue; the Sync HWDGE queue
            # carries only the xT chunk loads (keeps mm1's feed clear) ---
            w1_sb = consts.tile([P, CT, H], BF16)          # [c_p, c_tile, h]
            w1_re = w1bf.rearrange("(ct p) h -> p ct h", p=P)
            for ct in range(CT):
                nc.scalar.dma_start(w1_sb[:, ct:ct + 1, :],
                                    w1_re[:, ct:ct + 1, :])

            # ---- small constants on the SWDGE queue (off the Sync FIFO) --
            b1_sb = consts.tile([P, H], F32)               # b1 bcast rows
            nc.gpsimd.dma_start(b1_sb, b1d[:, :])
            w2_sb = consts.tile([P, H], BF16)              # w2 bcast rows
            nc.gpsimd.dma_start(w2_sb, w2bf[:, :])
            b2_sb = consts.tile([P, 1], F32)
            nc.gpsimd.dma_start(b2_sb, b2d[:, :])
            des_sb = consts.tile([P, BPC], F32)
            nc.gpsimd.dma_start(des_sb, desired[:, :])
            mask_sb = consts.tile([P, BPC, ST], F32)
            nc.gpsimd.dma_start(mask_sb, maskc.rearrange("b p j -> p b j"))

            iota0 = consts.tile([P, 512], F32)             # row = 0..511
            nc.gpsimd.iota(iota0, pattern=[[1, 512]], base=0,
                           channel_multiplier=0,
                           allow_small_or_imprecise_dtypes=True)
            iota1 = consts.tile([P, 512], F32)             # row = 1..512
            nc.gpsimd.iota(iota1, pattern=[[1, 512]], base=1,
                           channel_multiplier=0,
                           allow_small_or_imprecise_dtypes=True)

            # LT[k, m] = 1.0 iff k <= m  (inclusive lower-tri, csum matmul)
            pidx = consts.tile([P, 1], F32)                # pidx[p] = p
            nc.gpsimd.iota(pidx, pattern=[[1, 1]], base=0,
                           channel_multiplier=1,
                           allow_small_or_imprecise_dtypes=True)
            lt_sb = consts.tile([P, P], F32)
            nc.vector.tensor_scalar(lt_sb, iota0[:, :P], pidx, None, OP.is_ge)
            ones128 = consts.tile([P, P], F32)
            nc.vector.memset(ones128, 1.0)
            zeros_c = consts.tile([P, ST], F32)
            nc.vector.memset(zeros_c, 0.0)

            csum_cols = []
            prev_cols = []
            last_xt_inst = None

            # ================= phase 1: alpha / csum per sequence ========
            for b in range(BPC):
                # xT chunks (c_p=128, 512s): plain loads from the host-
                # transposed (C, S) copy of x -- s-block major for dep grain
                xTs = {}
                for sb in range(SB):
                    for ct in range(CT):
                        t_ = p_xT.tile([P, 512], BF16, tag="xT",
                                       name=f"xT_{b}_{sb}_{ct}")
                        last_xt_inst = nc.sync.dma_start(
                            t_, xcs[b, ct * P:(ct + 1) * P,
                                    sb * 512:(sb + 1) * 512])
                        xTs[(sb, ct)] = t_

                logit_c = p_cols.tile([P, ST], F32, tag="logit_c")

                # mm1 in (s, h) orientation: psum (128s, 512h) with
                # lhsT = xT slice, rhs = w1 slice; b1-add on DVE; tanh on
                # ACT; logits land directly as per-s-tile columns via DVE
                # multiply + free-dim reduce against broadcast w2
                for sb in range(SB):
                    for k in range(4):
                        st = 4 * sb + k
                        ksl = bass.ts(k, P)
                        h_sb = p_hT.tile([P, H], BF16, tag="hT")
                        for hh in range(2):
                            hsl = bass.ts(hh, 512)
                            ps = ps_mm1.tile([P, 512], F32, tag="mm1")
                            for ct in range(CT):
                                nc.tensor.matmul(
                                    ps,
                                    xTs[(sb, ct)][:, ksl],
                                    w1_sb[:, ct, hsl],
                                    start=(ct == 0), stop=(ct == CT - 1),
                                )
                            hpre = p_hpre.tile([P, 512], F32, tag="hpre")
                            nc.vector.tensor_add(hpre, ps, b1_sb[:, hsl])
                            nc.scalar.activation(h_sb[:, hsl], hpre,
                                                 ACTF.Tanh)
                        scr = p_scr.tile([P, H], F32, tag="scr")
                        nc.vector.tensor_mul(scr, h_sb, w2_sb)
                        nc.vector.tensor_reduce(
                            logit_c[:, st:st + 1], scr,
                            axis=mybir.AxisListType.X, op=OP.add)

                # ---- column space: sigmoid, mask, sum, scale, cumsum ----
                araw = p_cols.tile([P, ST], F32, tag="araw")
                nc.scalar.activation(araw, logit_c, ACTF.Sigmoid,
                                     bias=b2_sb[:, 0:1])
                nc.vector.tensor_mul(araw, araw, mask_sb[:, b, :])
                apart = p_cols.tile([P, 1], F32, tag="apart")
                nc.vector.tensor_reduce(apart, araw,
                                        axis=mybir.AxisListType.X, op=OP.add)
                sps = ps_small.tile([P, 1], F32, tag="smallps", name=f"sps{b}")
                nc.tensor.matmul(sps, ones128, apart, start=True, stop=True)
                asum = p_cols.tile([P, 1], F32, tag="asum")
                nc.vector.tensor_copy(asum, sps)
                nc.gpsimd.dma_start(asum_d[0:1, b:b + 1], asum[0:1, 0:1])
                rec = p_cols.tile([P, 1], F32, tag="rec")
                nc.vector.reciprocal(rec, asum)
                scl = p_cols.tile([P, 1], F32, tag="scl")
                nc.vector.tensor_mul(scl, rec, des_sb[:, b:b + 1])
                a_scl = p_cols.tile([P, ST], F32, tag="a_scl")
                nc.vector.tensor_scalar(a_scl, araw, scl, None, OP.mult)

                # inter-tile exclusive prefix (per partition, along tiles)
                bincl = p_cols.tile([P, ST], F32, tag="bincl")
                nc.vector.tensor_tensor_scan(bincl, a_scl, zeros_c, 0.0,
                                             OP.add, OP.add)
                bexcl = p_cols.tile([P, ST], F32, tag="bexcl")
                nc.vector.tensor_sub(bexcl, bincl, a_scl)

                # csum[p,j] = sum_{k<=p} a[k,j] + sum_k bexcl[k,j]
                cps = ps_small.tile([P, ST], F32, tag="smallps", name=f"cps{b}")
                nc.tensor.matmul(cps, lt_sb, a_scl, start=True, stop=False)
                nc.tensor.matmul(cps, ones128, bexcl, start=False, stop=True)
                csum_c = p_cols.tile([P, ST], F32, tag="csum_c")
                nc.vector.tensor_copy(csum_c, cps)
                prev_c = p_cols.tile([P, ST], F32, tag="prev_c")
                nc.vector.tensor_sub(prev_c, csum_c, a_scl)
                csum_cols.append(csum_c)
                prev_cols.append(prev_c)

            # ================= phase 2: W build + output matmul ==========
            for b in range(BPC):
                w_t = p_W.tile([P, ST, T], BF16, tag="W")
                for j in range(ST):
                    m2 = p_wtmp.tile([P, T], F32, tag="m2")
                    nc.vector.tensor_scalar(m2, iota0[:, :T],
                                            prev_cols[b][:, j:j + 1], None,
                                            OP.max)
                    wd = p_wtmp.tile([P, T], F32, tag="wd")
                    nc.vector.scalar_tensor_tensor(
                        wd, iota1[:, :T], csum_cols[b][:, j:j + 1], m2,
                        OP.min, OP.subtract)
                    nc.scalar.activation(w_t[:, j, :], wd, ACTF.Relu)

                for ch in range(2):
                    csl = bass.ts(ch, 512)
                    x_sc = p_xsc.tile([P, ST, 512], BF16, tag="x_sc")
                    xsc_inst = nc.gpsimd.dma_start(
                        x_sc,
                        xbf[b, :, csl].rearrange("(j p) c -> p j c", p=P))
                    if last_xt_inst is not None:
                        # keep x_sc traffic off the DMA queues until the
                        # mm1-feeding xT loads have been issued
                        tile_rust.add_dep_helper(
                            xsc_inst.ins, last_xt_inst.ins,
                            sync=True,
                            reason="delay x_sc until xT loads queued")
                    for tt in range(TT):
                        pso = ps_mm2.tile([P, 512], F32, tag="mm2",
                                          name=f"mm2_{b}_{ch}_{tt}")
                        for j in range(ST):
                            nc.tensor.matmul(
                                pso,
                                w_t[:, j, tt * P:(tt + 1) * P],
                                x_sc[:, j, :],
                                start=(j == 0), stop=(j == ST - 1),
                            )
                        ost = p_ost.tile([P, 512], F32, tag="ost")
                        nc.vector.tensor_copy(ost, pso)
                        nc.gpsimd.dma_start(
                            out_d[tt * P:(tt + 1) * P, b, csl], ost)

    nc.finalize()
    return nc


def _get_program():
    global _PROGRAM
    if _PROGRAM is None:
        _PROGRAM = _build_program()
    return _PROGRAM


def kernel(x, encoder_padding_mask, target_lengths, w1, b1, v2, g2, b2):
    global LAST_RESULTS
    bf = ml_dtypes.bfloat16

    x = np.asarray(x)
    mask = np.asarray(encoder_padding_mask)
    tl = np.asarray(target_lengths)
    w1 = np.asarray(w1, dtype=np.float32)
    b1 = np.asarray(b1, dtype=np.float32)
    v2 = np.asarray(v2, dtype=np.float32)
    g2 = np.asarray(g2, dtype=np.float32)
    b2 = np.asarray(b2, dtype=np.float32)

    # host-side marshaling: device layouts / dtypes
    xb = np.ascontiguousarray(x.transpose(1, 0, 2)).astype(bf)     # (B,S,C)
    xcs = np.ascontiguousarray(xb.transpose(0, 2, 1))              # (B,C,S)
    w1bf = w1.astype(bf)
    w2 = (np.float32(g2.reshape(())) * v2 /
          np.float32(np.linalg.norm(v2))).astype(np.float32)
    w2bf = np.ascontiguousarray(np.broadcast_to(w2.astype(bf), (P, H)))
    b2a = np.ascontiguousarray(
        np.broadcast_to(np.asarray(b2, np.float32).reshape(1, 1), (P, 1)))
    keepf = (~mask.astype(bool)).astype(np.float32)                # (B,S)
    # pre-columnized keep-mask: maskc[b, p, j] = keepf[b, j*128 + p]
    maskc = np.ascontiguousarray(
        keepf.reshape(B, ST, P).transpose(0, 2, 1))                # (B,P,ST)
    des = (np.float32(BETA) * tl.astype(np.float32) +
           np.float32(1e-4)).astype(np.float32)                    # (B,)

    nc = _get_program()
    in_maps = []
    for i in range(NCORES):
        sl = slice(i * BPC, (i + 1) * BPC)
        in_maps.append(dict(
            xbf=np.ascontiguousarray(xb[sl]),
            xcs=np.ascontiguousarray(xcs[sl]),
            w1bf=w1bf,
            b1d=np.ascontiguousarray(np.broadcast_to(b1, (P, H))),
            w2bf=w2bf,
            b2d=b2a,
            maskc=np.ascontiguousarray(maskc[sl]),
            desired=np.ascontiguousarray(
                np.broadcast_to(des[sl], (P, BPC))),
        ))

    res = run_bass_kernel_spmd(nc, in_maps, list(range(NCORES)))
    LAST_RESULTS = res

    out = np.concatenate([res.results[i]["out"] for i in range(NCORES)],
                         axis=1)                                   # (T,B,C)
    alpha_sum = np.concatenate(
        [res.results[i]["alpha_sum"].reshape(BPC) for i in range(NCORES)])
    feat_lengths = tl
    return out, feat_lengths, alpha_sum.astype(np.float32)


# revision 36
# speedup vs baseline: 1.3192x; 1.0689x over previous
"""CIF (continuous integrate-and-fire) layer as a Trainium2 Bass/Tile kernel.

Strategy
--------
Data-parallel over batch: B=16 sequences, 8 NeuronCores, 2 sequences/core.

The CIF scatter is reformulated as a dense banded matmul: sequence step s
carries mass over the interval [csum[s-1], csum[s]] and output slot t
integrates mass over [t, t+1), so the weight of x[s] on out[t] is the
interval overlap

    W[s, t] = clip(csum[s] - t, 0, 1) - clip(csum[s-1] - t, 0, 1)

which matches the reference's floor/scatter/cumsum formulation exactly for
all t < T (the reference's dump slots t >= T are dropped by out[:, :T]).
Then out_b = W_b^T @ x_b is a tensor-engine matmul.

Per-core pipeline (per sequence):
  1. xT chunks (128c x 512s, bf16) as plain DMA loads from a host-
     transposed (C, S) copy of x, issued per (s-block, c-tile) so mm1
     starts as soon as the first s-block's tiles land
  2. hT = tanh(w1^T @ xT + b1)       [PE + ACT, bf16]
  3. logits row (1, 512) per s-block [PE]; sigmoid+b2 [ACT]
  4. columnize alpha into (128, 16) via K=1 outer-product matmuls,
     overlapped per s-block
  5. all alpha math in 128-lane column space: mask (pre-columnized on
     host), alpha_sum via ones-matmul partition broadcast, scale,
     per-tile prefix via free-dim scan, cumsum via triangular matmul
  6. W tiles (128s x 512t) on DVE (min/max/sub) + relu-cast bf16 on ACT
  7. out (t, c) psum = sum_s W^T x   [PE, tt-outer so copies overlap]
"""

import os
import sys

import numpy as np

for _p in ("/root/.axon_site", "/root/.axon_site/_ro/trn_rl_repo",
           "/root/.axon_site/_ro/pypackages", "/opt/trn_rl_repo"):
    if os.path.isdir(_p) and _p not in sys.path:
        sys.path.append(_p)

import ml_dtypes  # noqa: E402

# The agent image's antenv package lacks axon_hooks; run_bass_kernel_spmd
# imports it when BASS_TRACE is set. Register a working shim so tracing
# degrades gracefully (and works when the libaxon profile ABI is present).
try:
    import antenv.axon_hooks  # noqa: F401
except ImportError:
    try:
        import types

        import antenv
        from trn_agent_boot.trn_boot import _ntff_profile_via_ctypes

        _hooks = types.ModuleType("antenv.axon_hooks")
        _hook_holder = [None]
        try:
            _hook_holder[0] = _ntff_profile_via_ctypes(
                "/opt/axon/libaxon_pjrt.so")
        except Exception:
            pass
        _hooks.get_axon_ntff_profile_hook = lambda: _hook_holder[0]
        _hooks.set_axon_ntff_profile_hook = (
            lambda h: _hook_holder.__setitem__(0, h))
        sys.modules["antenv.axon_hooks"] = _hooks
        antenv.axon_hooks = _hooks
    except Exception:
        pass

import concourse.bass as bass  # noqa: E402
from concourse import bacc  # noqa: E402
import concourse.tile as tile  # noqa: E402
from concourse import tile_rust  # noqa: E402
from concourse import mybir  # noqa: E402
from concourse.bass_utils import run_bass_kernel_spmd  # noqa: E402

B, S, C, H, T = 16, 2048, 1024, 1024, 512
BETA = 1.0
NCORES = 8
BPC = B // NCORES            # sequences per core = 2
P = 128
ST = S // P                  # s-tiles per sequence = 16
SB = S // 512                # 512-wide s-blocks = 4
CT = C // P                  # c-tiles = 8
HT = H // P                  # h-tiles = 8
TT = T // P                  # t-tiles = 4

F32 = mybir.dt.float32
BF16 = mybir.dt.bfloat16
OP = mybir.AluOpType
ACTF = mybir.ActivationFunctionType

_PROGRAM = None
LAST_RESULTS = None


def _build_program():
    nc = bacc.Bacc()

    xbf = nc.declare_dram_parameter("xbf", [BPC, S, C], BF16, isOutput=False)
    xcs = nc.declare_dram_parameter("xcs", [BPC, C, S], BF16, isOutput=False)
    w1bf = nc.declare_dram_parameter("w1bf", [C, H], BF16, isOutput=False)
    b1d = nc.declare_dram_parameter("b1d", [P, H], F32, isOutput=False)
    w2bf = nc.declare_dram_parameter("w2bf", [P, H], BF16, isOutput=False)
    b2d = nc.declare_dram_parameter("b2d", [P, 1], F32, isOutput=False)
    maskc = nc.declare_dram_parameter("maskc", [BPC, P, ST], F32, isOutput=False)
    desired = nc.declare_dram_parameter("desired", [P, BPC], F32, isOutput=False)

    out_d = nc.declare_dram_parameter("out", [T, BPC, C], F32, isOutput=True)
    asum_d = nc.declare_dram_parameter("alpha_sum", [1, BPC], F32, isOutput=True)

    with tile.TileContext(nc) as tc:
        with (
            tc.tile_pool(name="consts", bufs=1) as consts,
            tc.tile_pool(name="xT", bufs=4 * CT) as p_xT,
            tc.tile_pool(name="xsc", bufs=2) as p_xsc,
            tc.tile_pool(name="hT", bufs=2) as p_hT,
            tc.tile_pool(name="wmat", bufs=BPC) as p_W,
            tc.tile_pool(name="cols", bufs=2) as p_cols,
            tc.tile_pool(name="wtmp", bufs=3) as p_wtmp,
            tc.tile_pool(name="hpre", bufs=3) as p_hpre,
            tc.tile_pool(name="scr", bufs=2) as p_scr,
            tc.tile_pool(name="ostage", bufs=3) as p_ost,
            tc.tile_pool(name="ps_mm1", bufs=3, space="PSUM") as ps_mm1,
            tc.tile_pool(name="ps_small", bufs=2, space="PSUM") as ps_small,
            tc.tile_pool(name="ps_mm2", bufs=3, space="PSUM") as ps_mm2,
        ):
            # ---- w1 via the Scalar HWDGE queue; the Sync HWDGE queue
            # carries only the xT chunk loads (keeps mm1's feed clear) ---
            w1_sb = consts.tile([P, CT, H], BF16)          # [c_p, c_tile, h]
            w1_re = w1bf.rearrange("(ct p) h -> p ct h", p=P)
            for ct in range(CT):
                nc.scalar.dma_start(w1_sb[:, ct:ct + 1, :],
                                    w1_re[:, ct:ct + 1, :])

            # ---- small constants on the SWDGE queue (off the Sync FIFO) --
            b1_sb = consts.tile([P, H], F32)               # b1 bcast rows
            nc.gpsimd.dma_start(b1_sb, b1d[:, :])
            w2_sb = consts.tile([P, H], BF16)              # w2 bcast rows
            nc.gpsimd.dma_start(w2_sb, w2bf[:, :])
            b2_sb = consts.tile([P, 1], F32)
            nc.gpsimd.dma_start(b2_sb, b2d[:, :])
            des_sb = consts.tile([P, BPC], F32)
            nc.gpsimd.dma_start(des_sb, desired[:, :])
            mask_sb = consts.tile([P, BPC, ST], F32)
            nc.gpsimd.dma_start(mask_sb, maskc.rearrange("b p j -> p b j"))

            iota0 = consts.tile([P, 512], F32)             # row = 0..511
            nc.gpsimd.iota(iota0, pattern=[[1, 512]], base=0,
                           channel_multiplier=0,
                           allow_small_or_imprecise_dtypes=True)
            iota1 = consts.tile([P, 512], F32)             # row = 1..512
            nc.gpsimd.iota(iota1, pattern=[[1, 512]], base=1,
                           channel_multiplier=0,
                           allow_small_or_imprecise_dtypes=True)

            # LT[k, m] = 1.0 iff k <= m  (inclusive lower-tri, csum matmul)
            pidx = consts.tile([P, 1], F32)                # pidx[p] = p
            nc.gpsimd.iota(pidx, pattern=[[1, 1]], base=0,
                           channel_multiplier=1,
                           allow_small_or_imprecise_dtypes=True)
            lt_sb = consts.tile([P, P], F32)
            nc.vector.tensor_scalar(lt_sb, iota0[:, :P], pidx, None, OP.is_ge)
            ones128 = consts.tile([P, P], F32)
            nc.vector.memset(ones128, 1.0)
            zeros_c = consts.tile([P, ST], F32)
            nc.vector.memset(zeros_c, 0.0)

            csum_cols = []
            prev_cols = []
            last_xt_inst = None

            # ================= phase 1: alpha / csum per sequence ========
            for b in range(BPC):
                # xT chunks (c_p=128, 512s): plain loads from the host-
                # transposed (C, S) copy of x -- s-block major for dep grain
                xTs = {}
                for sb in range(SB):
                    for ct in range(CT):
                        t_ = p_xT.tile([P, 512], BF16, tag="xT",
                                       name=f"xT_{b}_{sb}_{ct}")
                        last_xt_inst = nc.sync.dma_start(
                            t_, xcs[b, ct * P:(ct + 1) * P,
                                    sb * 512:(sb + 1) * 512])
                        xTs[(sb, ct)] = t_

                logit_c = p_cols.tile([P, ST], F32, tag="logit_c")

                # mm1 in (s, h) orientation: psum (128s, 512h) with
                # lhsT = xT slice, rhs = w1 slice; b1-add on DVE; tanh on
                # ACT; logits land directly as per-s-tile columns via DVE
                # multiply + free-dim reduce against broadcast w2
                for sb in range(SB):
                    for k in range(4):
                        st = 4 * sb + k
                        ksl = bass.ts(k, P)
                        h_sb = p_hT.tile([P, H], BF16, tag="hT")
                        for hh in range(2):
                            hsl = bass.ts(hh, 512)
                            ps = ps_mm1.tile([P, 512], F32, tag="mm1")
                            for ct in range(CT):
                                nc.tensor.matmul(
                                    ps,
                                    xTs[(sb, ct)][:, ksl],
                                    w1_sb[:, ct, hsl],
                                    start=(ct == 0), stop=(ct == CT - 1),
                                )
                            hpre = p_hpre.tile([P, 512], F32, tag="hpre")
                            nc.vector.tensor_add(hpre, ps, b1_sb[:, hsl])
                            nc.scalar.activation(h_sb[:, hsl], hpre,
                                                 ACTF.Tanh)
                        scr = p_scr.tile([P, H], F32, tag="scr")
                        nc.vector.scalar_tensor_tensor(
                            scr, h_sb, 1.0, w2_sb, OP.mult, OP.mult,
                            accum_out=logit_c[:, st:st + 1])

                # ---- column space: sigmoid, mask, sum, scale, cumsum ----
                araw = p_cols.tile([P, ST], F32, tag="araw")
                nc.scalar.activation(araw, logit_c, ACTF.Sigmoid,
                                     bias=b2_sb[:, 0:1])
                nc.vector.tensor_mul(araw, araw, mask_sb[:, b, :])
                apart = p_cols.tile([P, 1], F32, tag="apart")
                nc.vector.tensor_reduce(apart, araw,
                                        axis=mybir.AxisListType.X, op=OP.add)
                sps = ps_small.tile([P, 1], F32, tag="smallps", name=f"sps{b}")
                nc.tensor.matmul(sps, ones128, apart, start=True, stop=True)
                asum = p_cols.tile([P, 1], F32, tag="asum")
                nc.vector.tensor_copy(asum, sps)
                nc.gpsimd.dma_start(asum_d[0:1, b:b + 1], asum[0:1, 0:1])
                rec = p_cols.tile([P, 1], F32, tag="rec")
                nc.vector.reciprocal(rec, asum)
                scl = p_cols.tile([P, 1], F32, tag="scl")
                nc.vector.tensor_mul(scl, rec, des_sb[:, b:b + 1])
                a_scl = p_cols.tile([P, ST], F32, tag="a_scl")
                nc.vector.tensor_scalar(a_scl, araw, scl, None, OP.mult)

                # inter-tile exclusive prefix (per partition, along tiles)
                bincl = p_cols.tile([P, ST], F32, tag="bincl")
                nc.vector.tensor_tensor_scan(bincl, a_scl, zeros_c, 0.0,
                                             OP.add, OP.add)
                bexcl = p_cols.tile([P, ST], F32, tag="bexcl")
                nc.vector.tensor_sub(bexcl, bincl, a_scl)

                # csum[p,j] = sum_{k<=p} a[k,j] + sum_k bexcl[k,j]
                cps = ps_small.tile([P, ST], F32, tag="smallps", name=f"cps{b}")
                nc.tensor.matmul(cps, lt_sb, a_scl, start=True, stop=False)
                nc.tensor.matmul(cps, ones128, bexcl, start=False, stop=True)
                csum_c = p_cols.tile([P, ST], F32, tag="csum_c")
                nc.vector.tensor_copy(csum_c, cps)
                prev_c = p_cols.tile([P, ST], F32, tag="prev_c")
                nc.vector.tensor_sub(prev_c, csum_c, a_scl)
                csum_cols.append(csum_c)
                prev_cols.append(prev_c)

            # ================= phase 2: W build + output matmul ==========
            for b in range(BPC):
                w_t = p_W.tile([P, ST, T], BF16, tag="W")
                for j in range(ST):
                    m2 = p_wtmp.tile([P, T], F32, tag="m2")
                    nc.vector.tensor_scalar(m2, iota0[:, :T],
                                            prev_cols[b][:, j:j + 1], None,
                                            OP.max)
                    wd = p_wtmp.tile([P, T], F32, tag="wd")
                    nc.vector.scalar_tensor_tensor(
                        wd, iota1[:, :T], csum_cols[b][:, j:j + 1], m2,
                        OP.min, OP.subtract)
                    nc.scalar.activation(w_t[:, j, :], wd, ACTF.Relu)

                for ch in range(2):
                    csl = bass.ts(ch, 512)
                    x_sc = p_xsc.tile([P, ST, 512], BF16, tag="x_sc")
                    xsc_inst = nc.gpsimd.dma_start(
                        x_sc,
                        xbf[b, :, csl].rearrange("(j p) c -> p j c", p=P))
                    if last_xt_inst is not None:
                        # keep x_sc traffic off the DMA queues until the
                        # mm1-feeding xT loads have been issued
                        tile_rust.add_dep_helper(
                            xsc_inst.ins, last_xt_inst.ins,
                            sync=True,
                            reason="delay x_sc until xT loads queued")
                    for tt in range(TT):
                        pso = ps_mm2.tile([P, 512], F32, tag="mm2",
                                          name=f"mm2_{b}_{ch}_{tt}")
                        for j in range(ST):
                            nc.tensor.matmul(
                                pso,
                                w_t[:, j, tt * P:(tt + 1) * P],
                                x_sc[:, j, :],
                                start=(j == 0), stop=(j == ST - 1),
                            )
                        ost = p_ost.tile([P, 512], F32, tag="ost")
                        nc.scalar.copy(ost, pso)
                        nc.gpsimd.dma_start(
                            out_d[tt * P:(tt + 1) * P, b, csl], ost)

    nc.finalize()
    return nc


def _get_program():
    global _PROGRAM
    if _PROGRAM is None:
        _PROGRAM = _build_program()
    return _PROGRAM


def kernel(x, encoder_padding_mask, target_lengths, w1, b1, v2, g2, b2):
    global LAST_RESULTS
    bf = ml_dtypes.bfloat16

    x = np.asarray(x)
    mask = np.asarray(encoder_padding_mask)
    tl = np.asarray(target_lengths)
    w1 = np.asarray(w1, dtype=np.float32)
    b1 = np.asarray(b1, dtype=np.float32)
    v2 = np.asarray(v2, dtype=np.float32)
    g2 = np.asarray(g2, dtype=np.float32)
    b2 = np.asarray(b2, dtype=np.float32)

    # host-side marshaling: device layouts / dtypes
    xb = np.ascontiguousarray(x.transpose(1, 0, 2)).astype(bf)     # (B,S,C)
    xcs = np.ascontiguousarray(xb.transpose(0, 2, 1))              # (B,C,S)
    w1bf = w1.astype(bf)
    w2 = (np.float32(g2.reshape(())) * v2 /
          np.float32(np.linalg.norm(v2))).astype(np.float32)
    w2bf = np.ascontiguousarray(np.broadcast_to(w2.astype(bf), (P, H)))
    b2a = np.ascontiguousarray(
        np.broadcast_to(np.asarray(b2, np.float32).reshape(1, 1), (P, 1)))
    keepf = (~mask.astype(bool)).astype(np.float32)                # (B,S)
    # pre-columnized keep-mask: maskc[b, p, j] = keepf[b, j*128 + p]
    maskc = np.ascontiguousarray(
        keepf.reshape(B, ST, P).transpose(0, 2, 1))                # (B,P,ST)
    des = (np.float32(BETA) * tl.astype(np.float32) +
           np.float32(1e-4)).astype(np.float32)                    # (B,)

    nc = _get_program()
    in_maps = []
    for i in range(NCORES):
        sl = slice(i * BPC, (i + 1) * BPC)
        in_maps.append(dict(
            xbf=np.ascontiguousarray(xb[sl]),
            xcs=np.ascontiguousarray(xcs[sl]),
            w1bf=w1bf,
            b1d=np.ascontiguousarray(np.broadcast_to(b1, (P, H))),
            w2bf=w2bf,
            b2d=b2a,
            maskc=np.ascontiguousarray(maskc[sl]),
            desired=np.ascontiguousarray(
                np.broadcast_to(des[sl], (P, BPC))),
        ))

    res = run_bass_kernel_spmd(nc, in_maps, list(range(NCORES)))
    LAST_RESULTS = res

    out = np.concatenate([res.results[i]["out"] for i in range(NCORES)],
                         axis=1)                                   # (T,B,C)
    alpha_sum = np.concatenate(
        [res.results[i]["alpha_sum"].reshape(BPC) for i in range(NCORES)])
    feat_lengths = tl
    return out, feat_lengths, alpha_sum.astype(np.float32)
